# revision 1
# baseline (speedup 1.0000x reference)
"""Trainium2 Bass kernel for nn_NodeGenerator (GNN message passing).

Strategy (8 NeuronCores, SPMD, no collectives):
  - Nodes sharded across cores (12500/core). Full node-feature table is
    replicated in each core's HBM, split into 4 quartile tables so that
    dma_gather's int16 indices can address any row.
  - Only candidate owner nodes (softmax class-0 > 0.5 and deg > 0)
    produce nonzero output rows, so only their incident edges are
    gathered; the mask itself is computed on host in float64 from the
    full edge set and applied on device.
  - Directed edges partitioned per 128-node window, grouped per
    4-window chunk and v-quartile, padded to per-window tile caps
    (shared across cores for SPMD). One dma_gather per (chunk,
    quartile) fetches X[v] rows (<=1024 indices per call, HW limit).
  - Per window: a one-hot matrix S (iota-compare, bf16, DVE) and
    accumulating bf16 PE matmuls G.T @ S give feature-major neighbor
    sums [64, 128] in fp32 PSUM.
  - Neighbor mean + the small MLP run feature-major: bf16 PE matmuls
    (fp32 PSUM) with fused fp32 bias/activation on ACT.
  - Per-core outputs [67, cols] + [1, cols] are assembled on host.
"""

import numpy as np

N = 100000
D = 64
CORES = 8
CHUNK = 512  # MLP column tile (psum free-dim limit for f32)
WPC = CHUNK // 128  # windows per chunk


def _derived(n=N, cores=CORES):
    npc = n // cores              # nodes per core
    win = (npc + 127) // 128      # 128-node windows per core
    npcp = win * 128              # padded nodes per core
    nq = 4
    vq = (n + nq - 1) // nq       # rows per quartile gather table
    return npc, win, npcp, nq, vq


def _host_prep(node_features, node_operations, edge_index, n=N, cores=CORES):
    import ml_dtypes
    bf16 = np.float16
    npc, win, npcp, nq, vq = _derived(n, cores)
    X = np.ascontiguousarray(np.asarray(node_features, dtype=np.float32))
    ops = np.asarray(node_operations, dtype=np.float32)
    ei = np.asarray(edge_index, dtype=np.int64)
    src, dst = ei[0], ei[1]
    U = np.concatenate([src, dst])
    V = np.concatenate([dst, src])

    deg = np.bincount(U, minlength=n).astype(np.int64)
    o = ops.astype(np.float64)
    e = np.exp(o - o.max(axis=1, keepdims=True))
    p0 = e[:, 0] / e.sum(axis=1)
    maskf = ((p0 > 0.5) & (deg > 0)).astype(np.float32)
    recip = (1.0 / np.maximum(deg, 1.0)).astype(np.float32)

    # Only candidate owners contribute nonzero output rows.
    keep = maskf[U] > 0
    U, V = U[keep], V[keep]

    core = U // npc
    ulf = U - core * npc
    w = ulf >> 7
    ulocal = (ulf & 127).astype(np.float32)
    q = V // vq
    vloc = V - q * vq

    gkey = (core * win + w) * nq + q
    order = np.argsort(gkey, kind="stable")
    gk_s = gkey[order]
    vloc_s = vloc[order]
    ul_s = ulocal[order]
    ngroups = cores * win * nq
    counts = np.bincount(gk_s, minlength=ngroups)
    starts = np.zeros(ngroups + 1, np.int64)
    np.cumsum(counts, out=starts[1:])
    within = np.arange(len(gk_s), dtype=np.int64) - starts[gk_s]

    # per-(window, quartile) tile caps: max over cores, >=1
    cmax = counts.reshape(cores, win, nq).max(axis=0)        # [win, nq]
    CQW = np.maximum(1, -(-cmax // 128)).astype(np.int64)    # tiles
    qbaseW = np.zeros((win, nq), np.int64)
    np.cumsum(CQW[:, :-1], axis=1, out=qbaseW[:, 1:])
    TTW = CQW.sum(axis=1)
    TTmax = int(TTW.max())

    # per-window q-major flat buffers
    idxbuf = np.zeros((cores, win, TTmax * 128), np.int16)
    ulbuf = np.full((cores, win, TTmax, 128), -1.0, np.float32)
    cw = gk_s // nq
    qs = gk_s % nq
    ws = cw % win
    cs_ = cw // win
    flatpos = qbaseW[ws, qs] * 128 + within
    idxbuf[cs_, ws, flatpos] = vloc_s.astype(np.int16)
    ulbuf.reshape(cores, win, TTmax * 128)[cs_, ws, flatpos] = ul_s

    # chunk-grouped gather stream: per chunk, per quartile, per window
    nchunks = -(-win // WPC)
    chunk_meta = []     # per chunk: (TTc, [(q, cbase, [(w, wb, cnt)])])
    idx_stream = []     # int16 pieces of [cnt*128]
    tot_tiles = 0
    for ci in range(nchunks):
        wlist = list(range(ci * WPC, min((ci + 1) * WPC, win)))
        qinfo = []
        tt = 0
        for qq in range(nq):
            cbase = tt
            winfo = []
            wb = 0
            for w_ in wlist:
                cnt = int(CQW[w_, qq])
                qb = int(qbaseW[w_, qq])
                idx_stream.append(
                    idxbuf[:, w_, qb * 128:(qb + cnt) * 128])  # [cores, .]
                winfo.append((w_, wb, cnt))
                wb += cnt
            qinfo.append((qq, cbase, winfo))
            tt += wb
        chunk_meta.append((tot_tiles, tt, qinfo))
        tot_tiles += tt

    idx_flat = np.concatenate(idx_stream, axis=1)           # [cores, TOT*128]
    TOT = tot_tiles
    idx16 = idx_flat.reshape(cores, TOT * 8, 16).transpose(0, 2, 1)
    idx16 = np.ascontiguousarray(np.tile(idx16, (1, 8, 1)))  # [cores,128,TOT*8]

    ulp = np.ascontiguousarray(
        ulbuf.transpose(0, 3, 1, 2)).astype(bf16)  # [cores, 128, win, TTmax]

    xq = []
    for qq in range(nq):
        t = X[qq * vq:(qq + 1) * vq]
        if t.shape[0] < vq:
            t = np.concatenate([t, np.zeros((vq - t.shape[0], D), np.float32)])
        xq.append(np.ascontiguousarray(t))

    xt = np.zeros((cores, D, npcp), np.float32)
    xt[:, :, :npc] = X.T.reshape(D, cores, npc).transpose(1, 0, 2)
    rec = np.zeros((cores, 1, npcp), np.float32)
    rec[:, 0, :npc] = recip.reshape(cores, npc)
    msk = np.zeros((cores, 1, npcp), np.float32)
    msk[:, 0, :npc] = maskf.reshape(cores, npc)

    return dict(CQW=CQW, qbaseW=qbaseW, TTW=TTW, TTmax=TTmax, TOT=TOT,
                chunk_meta=chunk_meta, idx16=idx16, ulp=ulp, xq=xq,
                xt=xt.astype(bf16), rec=rec.astype(bf16), msk=msk.astype(bf16))


def _build(prep, n=N, cores=CORES, phases=(1, 2)):
    from concourse import bacc, mybir, tile
    f32 = mybir.dt.float32
    bf16 = mybir.dt.float16
    i16 = mybir.dt.int16
    AF = mybir.ActivationFunctionType
    ALU = mybir.AluOpType

    npc, win, npcp, nq, vq = _derived(n, cores)
    CQW, qbaseW, TTW = prep["CQW"], prep["qbaseW"], prep["TTW"]
    TTmax, TOT, chunk_meta = prep["TTmax"], prep["TOT"], prep["chunk_meta"]
    nchunks = -(-npcp // CHUNK)
    chunks = [(i, min(CHUNK, npcp - i * CHUNK)) for i in range(nchunks)]

    nc = bacc.Bacc("TRN2", debug=False, num_swdge_queues=4)

    def din(name, shape, dt=f32):
        return nc.dram_tensor(name, shape, dt, kind="ExternalInput")

    xqh = [din(f"x{qq}", [vq, D]) for qq in range(nq)]
    idxh = din("idx", [128, TOT * 8], i16)
    ulh = din("ul", [128, win, TTmax], bf16)
    xth = din("xt", [D, npcp], bf16)
    rech = din("rec", [1, npcp], bf16)
    mskh = din("msk", [1, npcp], bf16)
    w1ah = din("w1a", [D, 128], bf16)
    w1bh = din("w1b", [D, 128], bf16)
    w2h = din("w2", [128, D], bf16)
    w3h = din("w3", [D, 67], bf16)
    p1h = din("p1", [D, 32], bf16)
    p2h = din("p2", [32, 1], bf16)
    b1h = din("b1", [128, 1])
    b2h = din("b2", [D, 1])
    b3h = din("b3", [67, 1])
    pb1h = din("pb1", [32, 1])
    pb2h = din("pb2", [1, 1])
    o67h = nc.dram_tensor("o67", [67, npcp], f32, kind="ExternalOutput")
    oph = nc.dram_tensor("op", [1, npcp], f32, kind="ExternalOutput")

    with tile.TileContext(nc) as tc:
        with (
            tc.tile_pool(name="const", bufs=1) as cpool,
            tc.tile_pool(name="nsum", bufs=1) as npool,
            tc.tile_pool(name="seg", bufs=2) as spool,
            tc.tile_pool(name="mlp", bufs=2) as mpool,
            tc.tile_pool(name="pseg", bufs=3, space="PSUM") as psseg,
            tc.tile_pool(name="pmlp", bufs=2, space="PSUM") as psmlp,
        ):
            iota = cpool.tile([128, TTmax, 128], bf16)
            nc.gpsimd.iota(iota[:], pattern=[[0, TTmax], [1, 128]], base=0,
                           channel_multiplier=0,
                           allow_small_or_imprecise_dtypes=True)
            ones = cpool.tile([1, 67], bf16)
            nc.vector.memset(ones[:], 1.0)

            def load_const(h, shape, dt=f32):
                nm = f"c_{h.name}"
                t = cpool.tile(shape, dt, name=nm, tag=nm)
                nc.sync.dma_start(t[:], h[:])
                return t

            w1a_t = load_const(w1ah, [D, 128], bf16)
            w1b_t = load_const(w1bh, [D, 128], bf16)
            w2_t = load_const(w2h, [128, D], bf16)
            w3_t = load_const(w3h, [D, 67], bf16)
            p1_t = load_const(p1h, [D, 32], bf16)
            p2_t = load_const(p2h, [32, 1], bf16)
            b1_t = load_const(b1h, [128, 1])
            b2_t = load_const(b2h, [D, 1])
            b3_t = load_const(b3h, [67, 1])
            pb1_t = load_const(pb1h, [32, 1])
            pb2_t = load_const(pb2h, [1, 1])
            idx_all = load_const(idxh, [128, TOT * 8], i16)
            ul_all = load_const(ulh, [128, win, TTmax], bf16)
            xt_all = load_const(xth, [D, npcp], bf16)
            rec_all = load_const(rech, [1, npcp], bf16)
            msk_all = load_const(mskh, [1, npcp], bf16)

            nsum_tiles = {}
            for ci, cs in chunks:
                nsum_tiles[ci] = npool.tile([D, cs], f32, tag=f"nsum{ci}",
                                            name=f"nsum{ci}")

            # ---- Phase 1: neighbor sums, chunk-grouped gathers ----
            gq = 0
            for (ci, cs) in (chunks if 1 in phases else []):
                chbase, TTc, qinfo = chunk_meta[ci]
                xg = spool.tile([128, TTc, D], f32, tag="xg")
                xgb = spool.tile([128, TTc, D], bf16, tag="xgb")
                for qq, cbase, winfo in qinfo:
                    blk = sum(cnt for _, _, cnt in winfo)
                    nparts = -(-blk // 8)
                    per = -(-blk // nparts)
                    for t0 in range(0, blk, per):
                        cqt = min(per, blk - t0)
                        tb = cbase + t0
                        ib = (chbase + tb) * 8
                        nc.gpsimd.dma_gather(
                            xg[:, tb:tb + cqt, :], xqh[qq][:],
                            idx_all[:, ib:ib + cqt * 8],
                            cqt * 128, cqt * 128, D,
                            queue_num=gq % 4)
                        gq += 1
                nc.scalar.copy(xgb[:], xg[:])
                for wi in range(len(qinfo[0][2])):
                    w_ = qinfo[0][2][wi][0]
                    TTw = int(TTW[w_])
                    S = spool.tile([128, TTw, 128], bf16, tag="S")
                    nc.vector.tensor_tensor(
                        out=S[:], in0=iota[:, :TTw, :],
                        in1=ul_all[:, w_, :TTw].broadcast_to([128, TTw, 128]),
                        op=ALU.is_equal)
                    ps = psseg.tile([D, 128], f32, tag="ps")
                    nmm = sum(cnt for _, _, winfo in qinfo
                              for (w2_, _, cnt) in winfo if w2_ == w_)
                    k = 0
                    for qq, cbase, winfo in qinfo:
                        wb, cnt = next((wb, cnt) for (w2_, wb, cnt) in winfo
                                       if w2_ == w_)
                        sb = int(qbaseW[w_, qq])
                        for j in range(cnt):
                            nc.tensor.matmul(
                                ps[:], lhsT=xgb[:, cbase + wb + j, :],
                                rhs=S[:, sb + j, :],
                                start=(k == 0), stop=(k == nmm - 1))
                            k += 1
                    nc.scalar.copy(
                        nsum_tiles[ci][:, wi * 128:(wi + 1) * 128], ps[:])

            # ---- Phase 2: neighbor mean + MLP + mask, feature-major ----
            for ci, cs in (chunks if 2 in phases else []):
                base = ci * CHUNK
                rb = psmlp.tile([D, cs], f32, tag="small")
                nc.tensor.matmul(rb[:], lhsT=ones[:, :D],
                                 rhs=rec_all[:, base:base + cs],
                                 start=True, stop=True)
                nmean = mpool.tile([D, cs], bf16, tag="nmean")
                nc.vector.tensor_tensor(out=nmean[:], in0=nsum_tiles[ci][:],
                                        in1=rb[:], op=ALU.mult)

                h1p = psmlp.tile([128, cs], f32, tag="big")
                nc.tensor.matmul(h1p[:], lhsT=w1a_t[:],
                                 rhs=xt_all[:, base:base + cs],
                                 start=True, stop=False)
                nc.tensor.matmul(h1p[:], lhsT=w1b_t[:], rhs=nmean[:],
                                 start=False, stop=True)
                h1 = mpool.tile([128, cs], bf16, tag="h1")
                nc.scalar.activation(out=h1[:], in_=h1p[:], func=AF.Relu,
                                     bias=b1_t[:], scale=1.0)

                h2p = psmlp.tile([D, cs], f32, tag="big")
                nc.tensor.matmul(h2p[:], lhsT=w2_t[:], rhs=h1[:],
                                 start=True, stop=True)
                h2 = mpool.tile([D, cs], bf16, tag="h2")
                nc.scalar.activation(out=h2[:], in_=h2p[:], func=AF.Relu,
                                     bias=b2_t[:], scale=1.0)

                gp = psmlp.tile([67, cs], f32, tag="big")
                nc.tensor.matmul(gp[:], lhsT=w3_t[:], rhs=h2[:],
                                 start=True, stop=True)
                g67 = mpool.tile([67, cs], f32, tag="g67")
                nc.scalar.activation(out=g67[:], in_=gp[:], func=AF.Identity,
                                     bias=b3_t[:], scale=1.0)
                g64b = mpool.tile([D, cs], bf16, tag="g64b")
                nc.scalar.copy(g64b[:], g67[:D, :])

                pp = psmlp.tile([32, cs], f32, tag="small")
                nc.tensor.matmul(pp[:], lhsT=p1_t[:], rhs=g64b[:],
                                 start=True, stop=True)
                pa = mpool.tile([32, cs], bf16, tag="pa")
                nc.scalar.activation(out=pa[:], in_=pp[:], func=AF.Relu,
                                     bias=pb1_t[:], scale=1.0)

                prp = psmlp.tile([1, cs], f32, tag="small")
                nc.tensor.matmul(prp[:], lhsT=p2_t[:], rhs=pa[:],
                                 start=True, stop=True)
                pr = mpool.tile([1, cs], f32, tag="pr")
                nc.scalar.activation(out=pr[:], in_=prp[:], func=AF.Sigmoid,
                                     bias=pb2_t[:], scale=1.0)
                prm = mpool.tile([1, cs], f32, tag="prm")
                nc.vector.tensor_tensor(out=prm[:], in0=pr[:],
                                        in1=msk_all[:, base:base + cs],
                                        op=ALU.mult)
                nc.sync.dma_start(oph[:, base:base + cs], prm[:])

                mbp = psmlp.tile([67, cs], f32, tag="small")
                nc.tensor.matmul(mbp[:], lhsT=ones[:],
                                 rhs=msk_all[:, base:base + cs],
                                 start=True, stop=True)
                mb = mpool.tile([67, cs], f32, tag="mb")
                nc.scalar.copy(mb[:], mbp[:])
                o67s = mpool.tile([67, cs], f32, tag="o67")
                nc.vector.tensor_tensor(out=o67s[:], in0=g67[:], in1=mb[:],
                                        op=ALU.mult)
                nc.sync.dma_start(o67h[:, base:base + cs], o67s[:])

    nc.compile()
    return nc


def _in_maps(prep, W1, b1, W2, b2, W3, b3, P1, pb1, P2, pb2,
             n=N, cores=CORES):
    import ml_dtypes
    bf16 = np.float16
    W1 = np.asarray(W1, np.float32)
    W3 = np.asarray(W3, np.float32)
    b3 = np.asarray(b3, np.float32)
    w3p = np.ascontiguousarray(np.concatenate([W3[:, 3:], W3[:, :3]], axis=1))
    b3p = np.concatenate([np.asarray(b3)[3:], np.asarray(b3)[:3]])
    shared = {
        "w1a": np.ascontiguousarray(W1[:D]).astype(bf16),
        "w1b": np.ascontiguousarray(W1[D:]).astype(bf16),
        "w2": np.asarray(W2, np.float32).astype(bf16),
        "w3": w3p.astype(bf16),
        "p1": np.asarray(P1, np.float32).astype(bf16),
        "p2": np.asarray(P2, np.float32).astype(bf16),
        "b1": np.asarray(b1, np.float32).reshape(-1, 1),
        "b2": np.asarray(b2, np.float32).reshape(-1, 1),
        "b3": b3p.astype(np.float32).reshape(-1, 1),
        "pb1": np.asarray(pb1, np.float32).reshape(-1, 1),
        "pb2": np.asarray(pb2, np.float32).reshape(-1, 1),
    }
    for qq, t in enumerate(prep["xq"]):
        shared[f"x{qq}"] = t
    maps = []
    for c in range(cores):
        m = dict(shared)
        m["idx"] = prep["idx16"][c]
        m["ul"] = prep["ulp"][c]
        m["xt"] = prep["xt"][c]
        m["rec"] = prep["rec"][c]
        m["msk"] = prep["msk"][c]
        maps.append(m)
    return maps


def _assemble(results, n=N, cores=CORES):
    npc, win, npcp, nq, vq = _derived(n, cores)
    out = np.zeros((n, D + 4), np.float32)
    for c, r in enumerate(results):
        o67 = r["o67"][:, :npc]
        op = r["op"][:, :npc]
        sl = slice(c * npc, (c + 1) * npc)
        out[sl, 0:3] = o67[D:D + 3].T
        out[sl, 3:3 + D] = o67[:D].T
        out[sl, 3 + D] = op[0]
    return out


def kernel(**inputs):
    from concourse.bass_utils import run_bass_kernel_spmd
    prep = _host_prep(inputs["node_features"], inputs["node_operations"],
                      inputs["edge_index"])
    nc = _build(prep)
    maps = _in_maps(prep, inputs["W1"], inputs["b1"], inputs["W2"],
                    inputs["b2"], inputs["W3"], inputs["b3"], inputs["P1"],
                    inputs["pb1"], inputs["P2"], inputs["pb2"])
    res = run_bass_kernel_spmd(nc, maps, core_ids=list(range(CORES)))
    return _assemble(res.results)



# revision 13
# speedup vs baseline: 1.9864x; 1.9864x over previous
"""Trainium2 Bass kernel for nn_NodeGenerator (GNN message passing).

Strategy (8 NeuronCores, SPMD, no collectives):
  - Only candidate nodes (softmax class-0 > 0.5 and deg > 0) produce
    nonzero output rows. Candidates are compacted per core (~1550 of
    12500), so both the neighbor-mean and the MLP run only on them.
  - The full node-feature table [100000, 64] f32 sits once in each
    core's HBM; dma_gather reads it through 4 quartile row-views so
    int16 indices stay in range.
  - Directed candidate edges are sorted by (quartile, u-window) and
    padded to 128-edge tiles per (quartile, window); gather calls take
    8 tiles (1024 indices, the HW cap) from one quartile.
  - Per tile: a one-hot matrix S (iota vs u-local compare, fp16, DVE)
    and an accumulating fp16 PE matmul give feature-major neighbor
    sums; all windows accumulate concurrently in resident PSUM tiles
    [64, 512] (4 windows each).
  - Neighbor mean + MLP run feature-major on the compacted columns
    only, interleaved with the tail of phase 1 for overlap.
  - Host assembles: device rows are scattered back to candidate ids,
    non-candidates stay zero.
"""

import numpy as np

N = 100000
D = 64
CORES = 8
NPC = N // CORES
VQ = 25000
NQ = 4
CALL_TILES = 8          # 1024 indices per dma_gather (HW limit)
SCHUNK = 32             # tiles per S-matrix build batch


def _host_prep(node_features, node_operations, edge_index):
    fp16 = np.float16
    X = np.ascontiguousarray(np.asarray(node_features, dtype=np.float32))
    ops = np.asarray(node_operations, dtype=np.float32)
    ei = np.asarray(edge_index, dtype=np.int64)
    U = np.concatenate([ei[0], ei[1]])
    V = np.concatenate([ei[1], ei[0]])

    deg = np.bincount(U, minlength=N).astype(np.int64)
    o = ops.astype(np.float64)
    e = np.exp(o - o.max(axis=1, keepdims=True))
    p0 = e[:, 0] / e.sum(axis=1)
    mask = (p0 > 0.5) & (deg > 0)
    recip = (1.0 / np.maximum(deg, 1.0)).astype(np.float32)

    # compact candidates per core
    rank = np.full(N, -1, np.int64)
    cand_ids = []
    for c in range(CORES):
        ids = np.where(mask[c * NPC:(c + 1) * NPC])[0] + c * NPC
        rank[ids] = np.arange(len(ids))
        cand_ids.append(ids)
    cmax = max(len(i) for i in cand_ids)
    NW = -(-cmax // 128)            # candidate windows per core
    CP = NW * 128                   # padded candidate columns

    keep = mask[U]
    Uk, Vk = U[keep], V[keep]
    ck = Uk // NPC
    r = rank[Uk]
    w = r >> 7
    ul = (r & 127).astype(np.float32)
    q = Vk // VQ
    vloc = (Vk - q * VQ).astype(np.int16)

    # per-(quartile, window) tile caps shared across cores (SPMD)
    cnt = np.zeros((CORES, NQ, NW), np.int64)
    np.add.at(cnt, (ck, q, w), 1)
    cap = cnt.max(axis=0)                         # [NQ, NW]
    tqw = -(-cap // 128)                          # tiles per (q, w)
    tqw[cap == 0] = 0
    tq = tqw.sum(axis=1)                          # tiles per quartile
    TA = int(tq.sum())
    qtb = np.zeros(NQ + 1, np.int64)
    np.cumsum(tq, out=qtb[1:])                    # quartile tile base
    wbase = np.zeros((NQ, NW), np.int64)          # global tile base of (q,w)
    for qq in range(NQ):
        wbase[qq, 0] = qtb[qq]
        np.cumsum(tqw[qq, :-1], out=wbase[qq, 1:])
        wbase[qq, 1:] += qtb[qq]

    # slot assignment: edge -> (global tile, slot)
    gkey = (ck * NQ + q) * NW + w
    order = np.argsort(gkey, kind="stable")
    gk = gkey[order]
    ngroups = CORES * NQ * NW
    counts = np.bincount(gk, minlength=ngroups)
    starts = np.zeros(ngroups + 1, np.int64)
    np.cumsum(counts, out=starts[1:])
    within = np.arange(len(gk), dtype=np.int64) - starts[gk]
    qs = (gk // NW) % NQ
    ws = gk % NW
    cs_ = gk // (NQ * NW)
    pos = wbase[qs, ws] * 128 + within

    idxbuf = np.zeros((CORES, TA * 128), np.int16)
    ulbuf = np.full((CORES, TA * 128), -1.0, np.float32)
    idxbuf[cs_, pos] = vloc[order]
    ulbuf[cs_, pos] = ul[order]

    # gather index stream: [CORES, 128, TA*8] (16-wrap, replicated x8)
    idx16 = idxbuf.reshape(CORES, TA * 8, 16).transpose(0, 2, 1)
    idx16 = np.ascontiguousarray(np.tile(idx16, (1, 8, 1)))
    # u-local planes: [CORES, 128(slot), TA]
    ulp = np.ascontiguousarray(
        ulbuf.reshape(CORES, TA, 128).transpose(0, 2, 1)).astype(fp16)

    # gather call list: per quartile, chunks of CALL_TILES tiles
    calls = []
    for qq in range(NQ):
        t0 = int(qtb[qq])
        for ts in range(0, int(tq[qq]), CALL_TILES):
            nt = min(CALL_TILES, int(tq[qq]) - ts)
            calls.append((qq, t0 + ts, ts, nt))

    # per-tile window; per-chunk last-tile for MLP interleave
    tile_win = np.zeros(TA, np.int64)
    for qq in range(NQ):
        for ww in range(NW):
            tile_win[wbase[qq, ww]:wbase[qq, ww] + tqw[qq, ww]] = ww
    last_tile = np.full(NW, -1, np.int64)
    for t in range(TA):
        last_tile[tile_win[t]] = max(last_tile[tile_win[t]], t)
    NCH = -(-NW // 4)
    chunk_last = np.zeros(NCH, np.int64)
    for p in range(NCH):
        chunk_last[p] = max(last_tile[4 * p:min(4 * p + 4, NW)])

    # dense per-candidate inputs, feature-major, padded to CP columns
    xt = np.zeros((CORES, D, CP), fp16)
    rec = np.zeros((CORES, D, CP), np.float32)
    for c in range(CORES):
        k = len(cand_ids[c])
        xt[c, :, :k] = X[cand_ids[c]].T.astype(fp16)
        rec[c, :, :k] = np.broadcast_to(recip[cand_ids[c]], (D, k))

    return dict(X=X, NW=NW, CP=CP, TA=TA, tq=tq, qtb=qtb, calls=calls,
                tile_win=tile_win, tqw=tqw, wbase=wbase,
                NCH=NCH, chunk_last=chunk_last,
                idx16=idx16, ulp=ulp, xt=xt, rec=rec, cand_ids=cand_ids)


def _build(prep):
    from concourse import bacc, mybir, tile
    f32 = mybir.dt.float32
    fp16 = mybir.dt.float16
    i16 = mybir.dt.int16
    AF = mybir.ActivationFunctionType
    ALU = mybir.AluOpType

    NW, CP, TA = prep["NW"], prep["CP"], prep["TA"]
    tq, qtb, calls = prep["tq"], prep["qtb"], prep["calls"]
    tile_win, tqw, wbase = prep["tile_win"], prep["tqw"], prep["wbase"]
    NCH, chunk_last = prep["NCH"], prep["chunk_last"]

    # contiguous matmul runs: one per (quartile, window) with tiles,
    # in global tile order; each runs in its own PSUM bank (start=True
    # zeroes the whole 2KB zero-region, so banks can host only one
    # accumulation group at a time)
    run_of_tile = {}
    runs = []
    for qq in range(NQ):
        for ww in range(NW):
            if tqw[qq, ww] == 0:
                continue
            t0 = int(wbase[qq, ww])
            r = dict(idx=len(runs), t0=t0, t1=t0 + int(tqw[qq, ww]) - 1,
                     win=ww, first=all(tqw[q2, ww] == 0 for q2 in range(qq)))
            runs.append(r)
            for t in range(r["t0"], r["t1"] + 1):
                run_of_tile[t] = r

    nc = bacc.Bacc("TRN2", debug=False, num_swdge_queues=4)

    def din(name, shape, dt=f32):
        return nc.dram_tensor(name, shape, dt, kind="ExternalInput")

    xallh = din("xall", [N, D])
    idxh = [din(f"idx{qq}", [128, int(tq[qq]) * 8], i16) for qq in range(NQ)]
    ulh = din("ul", [128, TA], fp16)
    xth = din("xt", [D, CP], fp16)
    rech = din("rec", [D, CP])
    w1ah = din("w1a", [D, 128], fp16)
    w1bh = din("w1b", [D, 128], fp16)
    w2h = din("w2", [128, D], fp16)
    w3h = din("w3", [D, 67], fp16)
    p1h = din("p1", [D, 32], fp16)
    p2h = din("p2", [32, 1], fp16)
    b1h = din("b1", [128, 1])
    b2h = din("b2", [D, 1])
    b3h = din("b3", [67, 1])
    pb1h = din("pb1", [32, 1])
    pb2h = din("pb2", [1, 1])
    iotah = din("iot", [128, SCHUNK, 128], fp16)
    o67h = nc.dram_tensor("o67", [67, CP], f32, kind="ExternalOutput")
    oph = nc.dram_tensor("op", [1, CP], f32, kind="ExternalOutput")

    with tile.TileContext(nc) as tc:
        with (
            tc.tile_pool(name="const", bufs=1) as cpool,
            tc.tile_pool(name="xg", bufs=4) as gpool,
            tc.tile_pool(name="smat", bufs=2) as spool,
            tc.tile_pool(name="mlp", bufs=2) as mpool,
            tc.tile_pool(name="pseg", bufs=1, space="PSUM") as psseg,
            tc.tile_pool(name="pmlp", bufs=2, space="PSUM") as psmlp,
        ):
            def load_const(h, shape, dt=f32):
                nm = f"c_{h.name}"
                t = cpool.tile(shape, dt, name=nm, tag=nm)
                nc.sync.dma_start(t[:], h[:])
                return t

            idx_t = [load_const(idxh[qq], [128, int(tq[qq]) * 8], i16)
                     for qq in range(NQ)]
            ul_t = load_const(ulh, [128, TA], fp16)
            xt_t = load_const(xth, [D, CP], fp16)
            rec_t = load_const(rech, [D, CP])
            w1a_t = load_const(w1ah, [D, 128], fp16)
            w1b_t = load_const(w1bh, [D, 128], fp16)
            w2_t = load_const(w2h, [128, D], fp16)
            w3_t = load_const(w3h, [D, 67], fp16)
            p1_t = load_const(p1h, [D, 32], fp16)
            p2_t = load_const(p2h, [32, 1], fp16)
            b1_t = load_const(b1h, [128, 1])
            b2_t = load_const(b2h, [D, 1])
            b3_t = load_const(b3h, [67, 1])
            pb1_t = load_const(pb1h, [32, 1])
            pb2_t = load_const(pb2h, [1, 1])

            iota = load_const(iotah, [128, SCHUNK, 128], fp16)

            xgb = cpool.tile([128, TA, D], fp16, name="xgb", tag="xgb")
            acc = cpool.tile([D, CP], f32, name="acc", tag="acc")
            # 4 rotating accumulators, each a full 2KB PSUM zero-region
            ps = [psseg.tile([D, 512], f32, name=f"ps{p}", tag=f"ps{p}")
                  for p in range(4)]

            # S chunks: within each quartile, batches of SCHUNK tiles
            schunks = []
            for qq in range(NQ):
                t0 = int(qtb[qq])
                for ts in range(0, int(tq[qq]), SCHUNK):
                    nt = min(SCHUNK, int(tq[qq]) - ts)
                    schunks.append((t0 + ts, nt))

            def emit_chunk(p):
                c0 = 512 * p
                cs = min(512, CP - c0)
                nmean = mpool.tile([D, 512], fp16, tag="nmean")
                nc.vector.tensor_tensor(
                    out=nmean[:, :cs], in0=acc[:, c0:c0 + cs],
                    in1=rec_t[:, c0:c0 + cs], op=ALU.mult)
                h1p = psmlp.tile([128, 512], f32, tag="big")
                nc.tensor.matmul(h1p[:, :cs], lhsT=w1a_t[:],
                                 rhs=xt_t[:, c0:c0 + cs],
                                 start=True, stop=False)
                nc.tensor.matmul(h1p[:, :cs], lhsT=w1b_t[:],
                                 rhs=nmean[:, :cs], start=False, stop=True)
                h1 = mpool.tile([128, 512], fp16, tag="h1")
                nc.scalar.activation(out=h1[:, :cs], in_=h1p[:, :cs],
                                     func=AF.Relu, bias=b1_t[:], scale=1.0)
                h2p = psmlp.tile([D, 512], f32, tag="big")
                nc.tensor.matmul(h2p[:, :cs], lhsT=w2_t[:], rhs=h1[:, :cs],
                                 start=True, stop=True)
                h2 = mpool.tile([D, 512], fp16, tag="h2")
                nc.scalar.activation(out=h2[:, :cs], in_=h2p[:, :cs],
                                     func=AF.Relu, bias=b2_t[:], scale=1.0)
                gp = psmlp.tile([67, 512], f32, tag="small")
                nc.tensor.matmul(gp[:, :cs], lhsT=w3_t[:], rhs=h2[:, :cs],
                                 start=True, stop=True)
                g67 = mpool.tile([67, 512], f32, tag="g67")
                nc.scalar.activation(out=g67[:, :cs], in_=gp[:, :cs],
                                     func=AF.Identity, bias=b3_t[:],
                                     scale=1.0)
                g64b = mpool.tile([D, 512], fp16, tag="g64b")
                nc.scalar.copy(g64b[:, :cs], g67[:D, :cs])
                pp = psmlp.tile([32, 512], f32, tag="small")
                nc.tensor.matmul(pp[:, :cs], lhsT=p1_t[:], rhs=g64b[:, :cs],
                                 start=True, stop=True)
                pa = mpool.tile([32, 512], fp16, tag="pa")
                nc.scalar.activation(out=pa[:, :cs], in_=pp[:, :cs],
                                     func=AF.Relu, bias=pb1_t[:], scale=1.0)
                prp = psmlp.tile([1, 512], f32, tag="small")
                nc.tensor.matmul(prp[:, :cs], lhsT=p2_t[:], rhs=pa[:, :cs],
                                 start=True, stop=True)
                pr = mpool.tile([1, 512], f32, tag="pr")
                nc.scalar.activation(out=pr[:, :cs], in_=prp[:, :cs],
                                     func=AF.Sigmoid, bias=pb2_t[:],
                                     scale=1.0)
                nc.sync.dma_start(o67h[:, c0:c0 + cs], g67[:, :cs])
                nc.sync.dma_start(oph[:, c0:c0 + cs], pr[:, :cs])

            emitted_s = set()
            next_chunk = 0
            mm_frontier = 0     # tiles below this have their matmul emitted
            gq = 0
            for (qq, tg, ts, nt) in calls:
                xg = gpool.tile([128, CALL_TILES, D], f32, tag="xg")
                nc.gpsimd.dma_gather(
                    xg[:, :nt, :], xallh[qq * VQ:(qq + 1) * VQ, :],
                    idx_t[qq][:, ts * 8:(ts + nt) * 8],
                    nt * 128, nt * 128, D, queue_num=gq % 4)
                gq += 1
                nc.scalar.copy(xgb[:, tg:tg + nt, :], xg[:, :nt, :])
                # S batches fully covered by gathers emitted so far
                for (s0, sn) in schunks:
                    if s0 in emitted_s or s0 + sn > tg + nt:
                        continue
                    emitted_s.add(s0)
                    S = spool.tile([128, SCHUNK, 128], fp16, tag="S",
                                   name=f"S{s0}")
                    nc.vector.tensor_tensor(
                        out=S[:, :sn, :], in0=iota[:, :sn, :],
                        in1=ul_t[:, s0:s0 + sn].broadcast_to([128, sn, 128]),
                        op=ALU.is_equal)
                    for t in range(s0, s0 + sn):
                        r = run_of_tile[t]
                        pz = ps[r["idx"] % 4]
                        nc.tensor.matmul(
                            pz[:, :128],
                            lhsT=xgb[:, t, :], rhs=S[:, t - s0, :],
                            start=(t == r["t0"]), stop=(t == r["t1"]))
                        if t == r["t1"]:
                            asl = acc[:, r["win"] * 128:(r["win"] + 1) * 128]
                            if r["first"]:
                                nc.scalar.copy(asl, pz[:, :128])
                            else:
                                nc.vector.tensor_tensor(
                                    out=asl, in0=asl, in1=pz[:, :128],
                                    op=ALU.add)
                    mm_frontier = max(mm_frontier, s0 + sn)
                # interleave finished MLP chunks for overlap
                while (next_chunk < NCH
                       and chunk_last[next_chunk] < mm_frontier):
                    emit_chunk(next_chunk)
                    next_chunk += 1
            while next_chunk < NCH:
                emit_chunk(next_chunk)
                next_chunk += 1

    nc.compile()
    return nc


def _in_maps(prep, W1, b1, W2, b2, W3, b3, P1, pb1, P2, pb2):
    fp16 = np.float16
    W1 = np.asarray(W1, np.float32)
    W3 = np.asarray(W3, np.float32)
    b3 = np.asarray(b3, np.float32)
    w3p = np.ascontiguousarray(np.concatenate([W3[:, 3:], W3[:, :3]], axis=1))
    b3p = np.concatenate([b3[3:], b3[:3]])
    iot = np.broadcast_to(np.arange(128, dtype=np.float16),
                          (128, SCHUNK, 128))
    shared = {
        "xall": prep["X"],
        "iot": np.ascontiguousarray(iot),
        "w1a": np.ascontiguousarray(W1[:D]).astype(fp16),
        "w1b": np.ascontiguousarray(W1[D:]).astype(fp16),
        "w2": np.asarray(W2, np.float32).astype(fp16),
        "w3": w3p.astype(fp16),
        "p1": np.asarray(P1, np.float32).astype(fp16),
        "p2": np.asarray(P2, np.float32).astype(fp16),
        "b1": np.asarray(b1, np.float32).reshape(-1, 1),
        "b2": np.asarray(b2, np.float32).reshape(-1, 1),
        "b3": b3p.astype(np.float32).reshape(-1, 1),
        "pb1": np.asarray(pb1, np.float32).reshape(-1, 1),
        "pb2": np.asarray(pb2, np.float32).reshape(-1, 1),
    }
    tq, qtb = prep["tq"], prep["qtb"]
    maps = []
    for c in range(CORES):
        m = dict(shared)
        for qq in range(NQ):
            m[f"idx{qq}"] = np.ascontiguousarray(
                prep["idx16"][c][:, qtb[qq] * 8:(qtb[qq] + tq[qq]) * 8])
        m["ul"] = prep["ulp"][c]
        m["xt"] = prep["xt"][c]
        m["rec"] = prep["rec"][c]
        maps.append(m)
    return maps


def _assemble(results, prep):
    out = np.zeros((N, D + 4), np.float32)
    for c, r in enumerate(results):
        ids = prep["cand_ids"][c]
        k = len(ids)
        o67 = r["o67"][:, :k]
        out[ids, 0:3] = o67[D:D + 3].T
        out[ids, 3:3 + D] = o67[:D].T
        out[ids, 3 + D] = r["op"][0, :k]
    return out


def kernel(**inputs):
    from concourse.bass_utils import run_bass_kernel_spmd
    prep = _host_prep(inputs["node_features"], inputs["node_operations"],
                      inputs["edge_index"])
    nc = _build(prep)
    maps = _in_maps(prep, inputs["W1"], inputs["b1"], inputs["W2"],
                    inputs["b2"], inputs["W3"], inputs["b3"], inputs["P1"],
                    inputs["pb1"], inputs["P2"], inputs["pb2"])
    res = run_bass_kernel_spmd(nc, maps, core_ids=list(range(CORES)))
    return _assemble(res.results, prep)
